# revision 4
# baseline (speedup 1.0000x reference)
"""Trainium2 Bass kernel for nn_OcclusionThirdLayer.

Reference computes out = W @ x + bias where W is a structured sparse
matrix: row r = i*224 + j has -1 at columns i*448 + j and i*448 + 224 + j,
and bias is all ones.  Equivalently, with x3 = x.reshape(32, 2, 224):

    out.reshape(32, 224)[i, j] = 1 - x3[i, 0, j] - x3[i, 1, j]

The matmul is skipped entirely (the 7168x14336 W is never touched).

Sharding: core c of 8 handles i-blocks [4c, 4c+4) -> a contiguous
1792-float slice of x in, a contiguous 896-float slice of out.  The
shard is host-rearranged to a [64, 28] tile (partition p = (i, chunk),
row = a-half | b-half) so the DVE op spreads over 64 partitions
(14 elems/partition) instead of 4x224.

Per-core program (raw Bass, no Tile):
  SP:  dma_start(tx <- x_shard)            .then_inc(dma_sem, 16)
  DVE: ty = (1 - A) - B                    [single STT via reverse0;
                                            wait dma_sem>=16 fused]
  SP:  dma_start(out_shard <- ty)          [wait v_sem>=1 fused]

Perf notes (HW-traced, NTFF window = first "useful" (compute-class)
instruction start -> last instruction end incl. the fixed ~6.9 us NRT
postamble of per-engine semaphore resets):
  - The bass-init constant memsets + initial all-engine barrier are
    stripped from the entry block: a memset would mark "useful" at
    t~0 and the barrier serialized the body behind the slowest
    engine's ~6.3 us NEFF preamble.
  - Single compute instruction: InstTensorScalarPtr with reverse0=1
    computes (1 - a) - b in one DVE op (bass doesn't expose reverse0;
    built directly).  walrus requires every DMA to carry a semaphore
    update, so both DMAs keep .then_inc.
  - 64-partition layout cuts the STT from ~380 ns (4 partitions) to
    ~165 ns.
  - Tail chain after compute start: sem hop ~30 + SP DGE-config ~555
    (single_packet=True on the out-DMA shaves ~20 ns off the trigger)
    + NRT drain (~237 ns odd cores / ~374 ns even cores - intrinsic
    LNC asymmetry) + barrier arrival; everything after (serpentine +
    51 sem resets/engine at ~118 ns each on Tensor + final barrier)
    is NRT-fixed at ~6.9 us.
  - Measured: ~8.15-8.20 us max across 8 cores (baseline 8.7 us); DMA
    triggers and DMA transfers are excluded from the window start, so
    the input DMA latency is off-window.  NRT postamble dominates;
    per-core true body is ~1.2 us.  Run-to-run sigma ~40 ns; other
    device activity during the traced run inflates the window.
"""

import numpy as np

N_CORES = 8
SIZE_IN = 14336
SIZE_OUT = 7168
BLOCK = 224          # j dimension
I_PER_CORE = 4       # i-blocks per core (32 total / 8 cores)
P = 64               # SBUF partitions used per core
F = 896 // P         # output floats per partition
CP = P // I_PER_CORE # chunks per i-block

_prog_cache = {}


def _ensure_axon_hooks_importable():
    """Some images ship an `antenv` without `axon_hooks`; bass_utils
    imports it unconditionally when tracing is requested. Install a
    no-op stub so a BASS_TRACE env var can't crash the run."""
    try:
        import antenv.axon_hooks  # noqa: F401
    except ImportError:
        import sys
        import types

        try:
            import antenv
        except ImportError:
            return
        stub = types.ModuleType("antenv.axon_hooks")
        stub._ntff_profile_hook = None

        def set_axon_ntff_profile_hook(hook):
            stub._ntff_profile_hook = hook

        def get_axon_ntff_profile_hook():
            return stub._ntff_profile_hook

        stub.set_axon_ntff_profile_hook = set_axon_ntff_profile_hook
        stub.get_axon_ntff_profile_hook = get_axon_ntff_profile_hook
        sys.modules["antenv.axon_hooks"] = stub
        antenv.axon_hooks = stub


def _strip_preamble(nc):
    """Drop bass-init const memsets, register-init moves and the initial
    all-engine barrier from the entry block. Must run right after Bass()
    construction, before any user instructions are added."""
    bb = nc.m.functions[0].blocks[0]
    keep = []
    for ins in bb.instructions:
        tn = type(ins).__name__
        if tn in ("InstMemset", "InstDrain", "InstEventSemaphore", "InstRegisterMove"):
            continue
        keep.append(ins)
    bb.instructions = keep


def _stt_reverse0(eng, out, in0, scalar, in1, op0, op1):
    """scalar_tensor_tensor with reverse0: out = (scalar op0 in0) op1 in1.
    bass's scalar_tensor_tensor computes (in0 op0 scalar) op1 in1; the ISA
    supports reversing op0's operands but bass doesn't expose it."""
    import concourse.mybir as mybir

    return eng.add_instruction(
        mybir.InstTensorScalarPtr(
            name=eng.bass.get_next_instruction_name(),
            is_scalar_tensor_tensor=True,
            op0=op0,
            op1=op1,
            reverse0=True,
            ins=[
                eng.lower_ap(in0),
                eng.lower_ap_or_imm(float(scalar)),
                eng.lower_ap(in1),
            ],
            outs=[eng.lower_ap(out)],
        )
    )


def _build_program():
    import concourse.bass as bass
    import concourse.mybir as mybir

    fp32 = mybir.dt.float32
    nc = bass.Bass(enable_partition_id=False)
    x_sh = nc.dram_tensor("x_shard", [P, 2 * F], fp32, kind="ExternalInput")
    out_sh = nc.dram_tensor("out_shard", [P, F], fp32, kind="ExternalOutput")

    _strip_preamble(nc)

    with (
        nc.sbuf_tensor("tx", [P, 2 * F], fp32) as tx,
        nc.sbuf_tensor("ty", [P, F], fp32) as ty,
        nc.semaphore("dma_sem") as dma_sem,
        nc.semaphore("v_sem") as v_sem,
    ):
        nc.sync.dma_start(tx[:], x_sh[:]).then_inc(dma_sem, 16)
        stt = _stt_reverse0(
            nc.vector, ty[:], tx[:, 0:F], 1.0, tx[:, F : 2 * F],
            mybir.AluOpType.subtract, mybir.AluOpType.subtract,
        )
        stt._wait_ge(dma_sem, 16)
        stt.then_inc(v_sem, 1)
        d_out = nc.sync.dma_start(
            out_sh[:], ty[:], single_packet=True).then_inc(dma_sem, 16)
        d_out._wait_ge(v_sem, 1)

    return nc


def _get_program():
    if "nc" not in _prog_cache:
        _ensure_axon_hooks_importable()
        _prog_cache["nc"] = _build_program()
    return _prog_cache["nc"]


def _shard(x):
    """x (14336,) -> 8 shards [P, 2F]: partition p = (i, chunk c),
    row = a-chunk | b-chunk."""
    shards = []
    for c in range(N_CORES):
        xc = x[c * 1792:(c + 1) * 1792].reshape(I_PER_CORE, 2, CP, F)
        shards.append(np.ascontiguousarray(
            xc.transpose(0, 2, 1, 3).reshape(P, 2 * F)))
    return shards


def _unshard(results):
    """Per-core [P, F] out tiles -> full (7168,) output; (i, c, j) rows
    are already in output order."""
    return np.concatenate(
        [results[c]["out_shard"].reshape(-1) for c in range(N_CORES)])


def kernel(x, W=None, bias=None, **_ignored):
    from concourse.bass_utils import run_bass_kernel_spmd

    x = np.ascontiguousarray(np.asarray(x, dtype=np.float32).reshape(SIZE_IN))
    nc = _get_program()
    in_maps = [{"x_shard": s} for s in _shard(x)]
    res = run_bass_kernel_spmd(nc, in_maps, list(range(N_CORES))).results
    return _unshard(res)
